# revision 31
# baseline (speedup 1.0000x reference)
"""NormLinearAttention Trainium2 kernel — 8-core sequence-parallel Bass/Tile.

Math (reference):
    q = k = elu(heads(x @ Wqk + bqk));  v = heads(silu(x @ Wv + bv))
    u = silu(x @ Wu + bu)
    kv[b,h] = k^T v  (contract over sequence);  att = q @ kv
    y = (u * layernorm(att)) @ Wo + bo

Sharding: each of 8 cores owns 512 tokens of each batch (2048 tokens total).
Per-core partial kv[b,h,d,e] is AllReduce-summed across cores in bf16 (2 MB),
overlapped with the u-projection; everything else is local.

v2 scheduling (vs the v1 baseline, measured on HW):
  - first-quarter wq/wv DMAs issue before the bulk xT load, so the PE starts
    ~35 us earlier.
  - phase 1 runs quarter-major passes (all 16 q tiles, then all 16 v tiles)
    so the ACT engine loads the Exp/Silu tables twice per quarter instead of
    twice per token tile (128 -> ~9 table loads, 1.3 us each).
  - q is spilled NATURAL and re-loaded transposed in phase 3 via the DMA
    xbar (dma_start_transpose), removing all 256 PE transpose matmuls and
    their PSUM/DVE copies.
  - the kv AllReduce runs in bf16 (verified 7.2e-3 rel err vs 2e-2 budget).
  - Wo streams in during phase 2 (u-projection) so phase 3 starts clean.
  - phase 3 is software-pipelined A(att+stats) / B(bcast) / Z(zT DVE chain) /
    Y(out-proj): each batch's 26 us zT vector chain hides under the previous
    batch's 67 us of output-projection matmuls, and y DMAs straight out of
    PSUM.
All matmuls bf16 (fp32 PSUM accumulate). fp8 was evaluated and rejected:
e4m3 operand noise gives 4-8e-2 rel err vs the 2e-2 gate (measured on the
fixed seed-0 inputs).
"""

import sys

if "/opt/trn_rl_repo" not in sys.path:
    sys.path.insert(0, "/opt/trn_rl_repo")

import numpy as np
import ml_dtypes

B, N, E = 4, 4096, 2048
H_DIM, HEADS, DH = 2048, 16, 128
N_CORES = 8
NL = N // N_CORES          # 512 tokens per (core, batch)
TL = B * NL                # 2048 local tokens per core
ET = E // 128              # 16 contraction tiles
JT = H_DIM // 128          # 16 hidden tiles
NJQ = 4                    # process hidden dim in 4 quarters of 512
TOK_B = NL // 128          # 4 token tiles per batch
LN_EPS = 1e-5

_BUILT = {}


def _build(flags, debug=False):
    import concourse.bacc as bacc
    import concourse.mybir as mybir
    import concourse.tile as tile

    has_bqv, has_bu, has_bo, has_affine = flags
    f32 = mybir.dt.float32
    bf16 = mybir.dt.bfloat16

    nc = bacc.Bacc("TRN2", target_bir_lowering=False, debug=False,
                   num_devices=N_CORES)

    t = {}
    t["xT"] = nc.dram_tensor("xT", [E, TL], bf16, kind="ExternalInput").ap()
    t["wqk"] = nc.dram_tensor("wqk", [E, H_DIM], bf16, kind="ExternalInput").ap()
    t["wv"] = nc.dram_tensor("wv", [E, H_DIM], bf16, kind="ExternalInput").ap()
    t["wu"] = nc.dram_tensor("wu", [E, H_DIM], bf16, kind="ExternalInput").ap()
    t["wo"] = nc.dram_tensor("wo", [H_DIM, E], bf16, kind="ExternalInput").ap()
    if has_bqv:
        t["bqk_r"] = nc.dram_tensor("bqk_r", [1, H_DIM], bf16,
                                    kind="ExternalInput").ap()
        t["bv_r"] = nc.dram_tensor("bv_r", [1, H_DIM], bf16,
                                   kind="ExternalInput").ap()
    if has_bo:
        t["bo_r"] = nc.dram_tensor("bo_r", [1, E], bf16,
                                   kind="ExternalInput").ap()
    if has_bu:
        t["bu_c"] = nc.dram_tensor("bu_c", [128, JT], f32,
                                   kind="ExternalInput").ap()
    if has_affine:
        t["g_c"] = nc.dram_tensor("g_c", [128, JT], f32,
                                  kind="ExternalInput").ap()
        t["b_c"] = nc.dram_tensor("b_c", [128, JT], f32,
                                  kind="ExternalInput").ap()
    t["y"] = nc.dram_tensor("y", [TL, E], f32, kind="ExternalOutput").ap()

    with tile.TileContext(nc) as tc:
        _body(nc, tc, tile, mybir, f32, bf16, t, flags)
    nc.compile()
    return nc


def _body(nc, tc, tile, mybir, f32, bf16, t, flags):
    Act = mybir.ActivationFunctionType
    Alu = mybir.AluOpType
    has_bqv, has_bu, has_bo, has_affine = flags

    with (
        tc.tile_pool(name="consts", bufs=1) as consts,
        tc.tile_pool(name="pre3", bufs=1) as pre3,
        tc.tile_pool(name="dram", bufs=1, space="DRAM") as dram,
    ):
        ones_col = consts.tile([128, 1], bf16)
        nc.vector.memset(ones_col, 1.0)
        ones_row = consts.tile([1, 128], bf16)
        nc.vector.memset(ones_row, 1.0)
        eps_sb = consts.tile([1, 1], f32)
        nc.vector.memset(eps_sb, LN_EPS)
        if has_bqv or has_bo:
            ones_bf = consts.tile([1, 128], bf16)
            nc.vector.memset(ones_bf, 1.0)
        if has_bqv:
            bqk_sb = consts.tile([1, H_DIM], bf16)
            nc.sync.dma_start(bqk_sb[:], t["bqk_r"][:])
            bv_sb = consts.tile([1, H_DIM], bf16)
            nc.sync.dma_start(bv_sb[:], t["bv_r"][:])
        if has_bo:
            bo_sb = consts.tile([1, E], bf16)
            nc.sync.dma_start(bo_sb[:], t["bo_r"][:])
        if has_bu:
            bu_sb = consts.tile([128, JT], f32)
            nc.sync.dma_start(bu_sb[:], t["bu_c"][:])
        if has_affine:
            g_sb = consts.tile([128, JT], f32)
            nc.sync.dma_start(g_sb[:], t["g_c"][:])
            b_sb = consts.tile([128, JT], f32)
            nc.sync.dma_start(b_sb[:], t["b_c"][:])

        q_dram = dram.tile([TL, H_DIM], bf16)
        uT_dram = dram.tile([H_DIM, TL], bf16)
        cc_in = dram.tile([B * HEADS * 128, DH], bf16)
        cc_out = dram.tile([B * HEADS * 128, DH], bf16, addr_space="Shared")

        with tc.tile_pool(name="xt_pool", bufs=1) as xt_pool:
            # w2 spans phases 1+2 (wu prefetch overlaps phase 1)
            w2_ctx = tc.tile_pool(name="w2", bufs=1)
            w2 = w2_ctx.__enter__()

            with (
                tc.tile_pool(name="w1", bufs=1) as w1,
                tc.tile_pool(name="st1", bufs=1) as st1,
                tc.tile_pool(name="ps_proj", bufs=1, space="PSUM") as psp,
                tc.tile_pool(name="ps_kv", bufs=1, space="PSUM") as pskv,
            ):
                # first-quarter weights BEFORE the bulk x load: the PE's
                # first matmul needs wq[jq=0] + xt chunk 0 only
                wq0 = w1.tile([128, ET, 512], bf16, tag="wq", bufs=1)
                nc.sync.dma_start(
                    wq0[:],
                    t["wqk"][:, 0:512].rearrange("(t p) j -> p t j", p=128))
                wv0 = w1.tile([128, ET, 512], bf16, tag="wv", bufs=1)
                nc.sync.dma_start(
                    wv0[:],
                    t["wv"][:, 0:512].rearrange("(t p) j -> p t j", p=128))

                xt = xt_pool.tile([128, ET, TL], bf16)   # 8 MB, ph1+2
                for tt in range(ET):
                    nc.sync.dma_start(
                        xt[:, tt],
                        t["xT"].rearrange("(t p) n -> t p n", p=128)[tt])

                wu_pre = []
                for jqu in range(2):
                    wu_q = w2.tile([128, ET, 512], bf16, tag="wu", bufs=2)
                    nc.sync.dma_start(
                        wu_q[:],
                        t["wu"][:, jqu * 512:(jqu + 1) * 512]
                        .rearrange("(t p) j -> p t j", p=128))
                    wu_pre.append(wu_q)

                # ---------------- phase 1: q/v projections + partial kv ----
                for jq in range(NJQ):
                    if jq == 0:
                        wq_sb, wv_sb = wq0, wv0
                    else:
                        wq_sb = w1.tile([128, ET, 512], bf16, tag="wq",
                                        bufs=1)
                        nc.sync.dma_start(
                            wq_sb[:],
                            t["wqk"][:, jq * 512:(jq + 1) * 512]
                            .rearrange("(t p) j -> p t j", p=128))
                        wv_sb = w1.tile([128, ET, 512], bf16, tag="wv",
                                        bufs=1)
                        nc.sync.dma_start(
                            wv_sb[:],
                            t["wv"][:, jq * 512:(jq + 1) * 512]
                            .rearrange("(t p) j -> p t j", p=128))

                    q_bf = st1.tile([128, 16, 512], bf16, tag="qbf", bufs=1)
                    v_bf = st1.tile([128, 16, 512], bf16, tag="vbf", bufs=1)

                    # q pass: 16 token tiles, Exp table loads once
                    for idx in range(16):
                        tok0 = idx * 128
                        q_ps = psp.tile([128, 512], f32, tag="qps", bufs=2)
                        for tt in range(ET):
                            nc.tensor.matmul(
                                q_ps[:], xt[:, tt, tok0:tok0 + 128],
                                wq_sb[:, tt],
                                start=(tt == 0),
                                stop=(not has_bqv and tt == ET - 1))
                        if has_bqv:
                            nc.tensor.matmul(
                                q_ps[:], ones_bf[:],
                                bqk_sb[:, jq * 512:(jq + 1) * 512],
                                start=False, stop=True)
                        # elu(q) = (max(q,0) - 1) + exp(min(q, 0))
                        tmin = st1.tile([128, 512], f32, tag="tmin", bufs=2)
                        nc.vector.tensor_scalar_min(tmin[:], q_ps[:], 0.0)
                        texp = st1.tile([128, 512], bf16, tag="texp", bufs=2)
                        nc.scalar.activation(texp[:], tmin[:], Act.Exp)
                        trelu = st1.tile([128, 512], bf16, tag="trelu",
                                         bufs=2)
                        nc.vector.tensor_scalar(trelu[:], q_ps[:], 0.0,
                                                -1.0, Alu.max, Alu.add)
                        nc.vector.tensor_add(q_bf[:, idx], trelu[:], texp[:])
                        # spill q natural; phase 3 re-loads it transposed
                        nc.sync.dma_start(
                            q_dram[tok0:tok0 + 128,
                                   jq * 512:(jq + 1) * 512],
                            q_bf[:, idx])

                    # v pass: Silu table loads once
                    for idx in range(16):
                        tok0 = idx * 128
                        v_ps = psp.tile([128, 512], f32, tag="vps", bufs=2)
                        for tt in range(ET):
                            nc.tensor.matmul(
                                v_ps[:], xt[:, tt, tok0:tok0 + 128],
                                wv_sb[:, tt],
                                start=(tt == 0),
                                stop=(not has_bqv and tt == ET - 1))
                        if has_bqv:
                            nc.tensor.matmul(
                                v_ps[:], ones_bf[:],
                                bv_sb[:, jq * 512:(jq + 1) * 512],
                                start=False, stop=True)
                        nc.scalar.activation(v_bf[:, idx], v_ps[:], Act.Silu)

                    # kv pass: per-head PSUM accumulation over token tiles
                    for b in range(B):
                        kv_sb = st1.tile([128, 4, DH], bf16, tag="kvsb",
                                         bufs=2)
                        for h in range(4):
                            kv_ps = pskv.tile([128, DH], f32, tag="kv",
                                              bufs=2)
                            for tk in range(TOK_B):
                                idx = b * 4 + tk
                                nc.tensor.matmul(
                                    kv_ps[:],
                                    q_bf[:, idx, h * 128:(h + 1) * 128],
                                    v_bf[:, idx, h * 128:(h + 1) * 128],
                                    start=(tk == 0), stop=(tk == TOK_B - 1))
                            nc.vector.tensor_copy(kv_sb[:, h], kv_ps[:])
                        r0 = (b * HEADS + jq * 4) * 128
                        nc.sync.dma_start(
                            cc_in[r0:r0 + 512, :]
                            .rearrange("(h d) e -> d h e", h=4),
                            kv_sb[:])

            # ---------------- kv AllReduce across the 8 cores (bf16) -------
            nc.gpsimd.collective_compute(
                "AllReduce", mybir.AluOpType.add,
                replica_groups=[list(range(N_CORES))],
                ins=[cc_in.opt()], outs=[cc_out.opt()])

            # ---------------- phase 2: uT projection (overlaps AR) ---------
            # batch 0's qT tiles stream in mid-phase-2 via the Activation
            # DGE queue (the Sync queue is blocked behind phase-2-dependent
            # u spills, and issuing at phase-2 start would contend with the
            # AllReduce's bandwidth peak): pre3 addresses don't overlap any
            # live pool and q_dram is final, so A(0) starts with operands
            # resident.
            qpre = {}

            with (
                tc.tile_pool(name="ps_u", bufs=1, space="PSUM") as psu,
            ):
                for jqu in range(NJQ):
                    if jqu == 2:
                        for h in range(HEADS):
                            qb = pre3.tile([128, NL], bf16, tag="qb",
                                           bufs=24)
                            nc.scalar.dma_start_transpose(
                                qb[:], q_dram[0:NL, h * 128:(h + 1) * 128])
                            qpre[(0, h)] = qb
                    if jqu < 2:
                        wu_q = wu_pre[jqu]
                    else:
                        wu_q = w2.tile([128, ET, 512], bf16, tag="wu", bufs=2)
                        nc.sync.dma_start(
                            wu_q[:],
                            t["wu"][:, jqu * 512:(jqu + 1) * 512]
                            .rearrange("(t p) j -> p t j", p=128))
                    for jl in range(4):
                        jt = jqu * 4 + jl
                        u_ps = psu.tile([128, 4, 512], f32, tag="ups", bufs=2)
                        for tt in range(ET):
                            for c in range(4):
                                nc.tensor.matmul(
                                    u_ps[:, c],
                                    wu_q[:, tt, jl * 128:(jl + 1) * 128],
                                    xt[:, tt, c * 512:(c + 1) * 512],
                                    start=(tt == 0), stop=(tt == ET - 1))
                        u_st = w2.tile([128, TL], bf16, tag="ust", bufs=1)
                        ubias = bu_sb[:, jt:jt + 1] if has_bu else 0.0
                        for c in range(4):
                            nc.scalar.activation(
                                u_st[:, c * 512:(c + 1) * 512], u_ps[:, c],
                                Act.Silu, bias=ubias)
                        nc.sync.dma_start(
                            uT_dram[jt * 128:(jt + 1) * 128, :], u_st[:])
                # batch 0's kv load: the ACT queue reaches this at the end
                # of the u-silu stream; the AllReduce is long finished
                kvb0 = pre3.tile([128, HEADS, DH], bf16, tag="kvb", bufs=2)
                nc.scalar.dma_start(
                    kvb0[:],
                    cc_out[0:HEADS * 128, :]
                    .rearrange("(h d) e -> d h e", h=HEADS))
            w2_ctx.__exit__(None, None, None)

            # ------------- phase 3: attention, layernorm, output proj ------
            # x is dead after phase 2 and xt has exactly wo's shape
            # [128, 16, 2048] bf16 — stream wo into it (zero extra SBUF).
            # Software pipeline: A(b)=att+stats (interleaved matmuls),
            # B(b)=stat broadcast, Z(b)=zT DVE chain, Y(b)=out-proj.
            # Z(b) hides under Y(b-1)'s matmul stream.
            wo_sb = xt
            with (
                tc.tile_pool(name="st3", bufs=1) as st3,
                tc.tile_pool(name="ps_att", bufs=1, space="PSUM") as psa,
                tc.tile_pool(name="ps_sm", bufs=1, space="PSUM") as pssm,
                tc.tile_pool(name="ps_y", bufs=1, space="PSUM") as psy,
            ):
                state = {}

                def wo_quarter(i):
                    for ct in range(i * 4, i * 4 + 4):
                        nc.sync.dma_start(
                            wo_sb[:, ct],
                            t["wo"].rearrange("(t p) e -> t p e", p=128)[ct])

                def att_block(b):
                    if b == 0:
                        kvb = kvb0
                    else:
                        kvb = pre3.tile([128, HEADS, DH], bf16, tag="kvb",
                                        bufs=2)
                        nc.sync.dma_start(
                            kvb[:],
                            cc_out[b * HEADS * 128:(b + 1) * HEADS * 128, :]
                            .rearrange("(h d) e -> d h e", h=HEADS))
                    att = st3.tile([128, HEADS, NL], bf16, tag="att", bufs=4)
                    sum_ps = pssm.tile([1, NL], f32, tag="sum", bufs=1)
                    ssq_ps = pssm.tile([1, NL], f32, tag="ssq", bufs=1)

                    # per-head: load qT via DMA xbar transpose, att matmul,
                    # then interleave the sum/ssq accumulation matmuls two
                    # heads behind so PE consumption paces the qb DMAs
                    def stats(h):
                        nc.tensor.matmul(sum_ps[:], ones_col[:], att[:, h],
                                         start=(h == 0),
                                         stop=(h == HEADS - 1))
                        sq = st3.tile([128, NL], bf16, tag="sq", bufs=1)
                        nc.vector.tensor_mul(sq[:], att[:, h], att[:, h])
                        nc.tensor.matmul(ssq_ps[:], ones_col[:], sq[:],
                                         start=(h == 0),
                                         stop=(h == HEADS - 1))

                    for h in range(HEADS):
                        if (b, h) in qpre:
                            qb = qpre[(b, h)]
                        else:
                            qb = pre3.tile([128, NL], bf16, tag="qb",
                                           bufs=24)
                            nc.sync.dma_start_transpose(
                                qb[:],
                                q_dram[b * NL:(b + 1) * NL,
                                       h * 128:(h + 1) * 128])
                        att_ps = psa.tile([128, NL], f32, tag="attps",
                                          bufs=2)
                        nc.tensor.matmul(att_ps[:], kvb[:, h], qb[:],
                                         start=True, stop=True)
                        nc.scalar.copy(att[:, h], att_ps[:])
                        if h >= 2:
                            stats(h - 2)
                    stats(HEADS - 2)
                    stats(HEADS - 1)

                    mean = st3.tile([1, NL], f32, tag="mean", bufs=1)
                    nc.vector.tensor_scalar_mul(mean[:], sum_ps[:],
                                                1.0 / H_DIM)
                    msq = st3.tile([1, NL], f32, tag="msq", bufs=1)
                    nc.vector.tensor_scalar_mul(msq[:], ssq_ps[:],
                                                1.0 / H_DIM)
                    m2 = st3.tile([1, NL], f32, tag="m2", bufs=1)
                    nc.vector.tensor_mul(m2[:], mean[:], mean[:])
                    nc.vector.tensor_sub(msq[:], msq[:], m2[:])   # var
                    nc.scalar.activation(m2[:], msq[:], Act.Sqrt,  # std
                                         bias=eps_sb[:])
                    rstd_f = st3.tile([1, NL], f32, tag="rstdf", bufs=1)
                    nc.vector.reciprocal(rstd_f[:], m2[:])
                    rstd = st3.tile([1, NL], bf16, tag="rstd", bufs=1)
                    nc.vector.tensor_copy(rstd[:], rstd_f[:])
                    mr = st3.tile([1, NL], bf16, tag="mr", bufs=1)
                    nc.vector.tensor_mul(mr[:], mean[:], rstd_f[:])
                    state[b] = (att, rstd, mr)

                def bcast_block(b):
                    att, rstd, mr = state[b]
                    # uT is only needed by z_block — loading it here keeps
                    # the phase-2-gated DMA waits off att_block's load path
                    uT_b = []
                    for half in range(2):
                        uh = st3.tile([128, 8, NL], bf16, tag="utb", bufs=2)
                        nc.sync.dma_start(
                            uh[:],
                            uT_dram[half * 1024:(half + 1) * 1024,
                                    b * NL:(b + 1) * NL]
                            .rearrange("(jt p) n -> p jt n", p=128))
                        uT_b.append(uh)
                    bc_ps = pssm.tile([128, 2, NL], f32, tag="bc", bufs=1)
                    nc.tensor.matmul(bc_ps[:, 0], ones_row[:], rstd[:],
                                     start=True, stop=True)
                    nc.tensor.matmul(bc_ps[:, 1], ones_row[:], mr[:],
                                     start=True, stop=True)
                    bc_sb = st3.tile([128, 2, NL], bf16, tag="bcs", bufs=2)
                    nc.vector.tensor_copy(bc_sb[:], bc_ps[:])
                    state[b] = (att, uT_b, bc_sb)

                def z_block(b):
                    # zT overwrites att in place: each att slice is fully
                    # consumed (stats + s1) before its zT write
                    att, uT_b, bc_sb = state[b]
                    for jt in range(JT):
                        s1 = st3.tile([128, NL], bf16, tag="s1", bufs=2)
                        nc.vector.tensor_mul(s1[:], att[:, jt], bc_sb[:, 0])
                        s2 = st3.tile([128, NL], bf16, tag="s2", bufs=2)
                        nc.vector.tensor_sub(s2[:], s1[:], bc_sb[:, 1])
                        if has_affine:
                            s3 = st3.tile([128, NL], bf16, tag="s3", bufs=2)
                            nc.vector.tensor_scalar(
                                s3[:], s2[:], g_sb[:, jt:jt + 1],
                                b_sb[:, jt:jt + 1], Alu.mult, Alu.add)
                        else:
                            s3 = s2
                        nc.vector.tensor_mul(att[:, jt], s3[:],
                                             uT_b[jt // 8][:, jt % 8])
                    state[b] = att

                def y_block(b):
                    zT = state.pop(b)
                    for tsl in range(TOK_B):
                        for eb in range(4):
                            e0 = eb * 512
                            y_ps = psy.tile([128, 512], f32, tag="yps",
                                            bufs=2)
                            for ct in range(JT):
                                nc.tensor.matmul(
                                    y_ps[:],
                                    zT[:, ct, tsl * 128:(tsl + 1) * 128],
                                    wo_sb[:, ct, e0:e0 + 512],
                                    start=(ct == 0),
                                    stop=(not has_bo and ct == JT - 1))
                            if has_bo:
                                nc.tensor.matmul(
                                    y_ps[:], ones_bf[:],
                                    bo_sb[:, e0:e0 + 512],
                                    start=False, stop=True)
                            ybuf = st3.tile([128, 512], f32, tag="ybuf",
                                            bufs=1)
                            nc.scalar.copy(ybuf[:], y_ps[:])
                            nc.sync.dma_start(
                                t["y"][b * NL + tsl * 128:
                                       b * NL + (tsl + 1) * 128,
                                       e0:e0 + 512],
                                ybuf[:])

                att_block(0)
                att_block(1)
                bcast_block(0)
                z_block(0)
                att_block(2)
                wo_quarter(0)
                bcast_block(1)
                z_block(1)
                wo_quarter(1)
                wo_quarter(2)
                wo_quarter(3)
                bcast_block(2)
                y_block(0)
                z_block(2)
                att_block(3)
                y_block(1)
                bcast_block(3)
                z_block(3)
                y_block(2)
                y_block(3)


def _get_nc(flags, debug=False):
    key = (flags, debug)
    if key not in _BUILT:
        _BUILT[key] = _build(flags, debug)
    return _BUILT[key]


def make_in_maps(x, Wqk, bqk, Wv, bv, Wu, bu, Wo, bo, ln_g, ln_b):
    bf16 = ml_dtypes.bfloat16
    f32 = np.float32
    x = np.asarray(x)
    flags = (
        bool(np.any(bqk) or np.any(bv)),
        bool(np.any(bu)),
        bool(np.any(bo)),
        bool(np.any(np.asarray(ln_g) != 1.0) or np.any(ln_b)),
    )
    shared = {
        "wqk": np.asarray(Wqk, f32).astype(bf16),
        "wv": np.asarray(Wv, f32).astype(bf16),
        "wu": np.asarray(Wu, f32).astype(bf16),
        "wo": np.asarray(Wo, f32).astype(bf16),
    }
    if flags[0]:
        shared["bqk_r"] = np.asarray(bqk, f32).astype(bf16).reshape(1, H_DIM)
        shared["bv_r"] = np.asarray(bv, f32).astype(bf16).reshape(1, H_DIM)
    if flags[1]:
        shared["bu_c"] = np.ascontiguousarray(
            np.asarray(bu, f32).reshape(JT, 128).T)
    if flags[2]:
        shared["bo_r"] = np.asarray(bo, f32).astype(bf16).reshape(1, E)
    if flags[3]:
        shared["g_c"] = np.ascontiguousarray(
            np.asarray(ln_g, f32).reshape(JT, 128).T)
        shared["b_c"] = np.ascontiguousarray(
            np.asarray(ln_b, f32).reshape(JT, 128).T)
    in_maps = []
    for c in range(N_CORES):
        xc = np.ascontiguousarray(
            x[:, c * NL:(c + 1) * NL, :].reshape(TL, E).T).astype(bf16)
        in_maps.append({"xT": xc, **shared})
    return flags, in_maps


def kernel(x, Wqk, bqk, Wv, bv, Wu, bu, Wo, bo, ln_g, ln_b, **_unused):
    from concourse.bass_utils import run_bass_kernel_spmd

    flags, in_maps = make_in_maps(x, Wqk, bqk, Wv, bv, Wu, bu, Wo, bo,
                                  ln_g, ln_b)
    nc = _get_nc(flags)
    res = run_bass_kernel_spmd(nc, in_maps, core_ids=list(range(N_CORES)))

    y = np.empty((B, N, E), np.float32)
    for c in range(N_CORES):
        y[:, c * NL:(c + 1) * NL, :] = res.results[c]["y"].reshape(B, NL, E)
    return y


# revision 32
# speedup vs baseline: 1.0166x; 1.0166x over previous
"""NormLinearAttention Trainium2 kernel — 8-core sequence-parallel Bass/Tile.

Math (reference):
    q = k = elu(heads(x @ Wqk + bqk));  v = heads(silu(x @ Wv + bv))
    u = silu(x @ Wu + bu)
    kv[b,h] = k^T v  (contract over sequence);  att = q @ kv
    y = (u * layernorm(att)) @ Wo + bo

Sharding: each of 8 cores owns 512 tokens of each batch (2048 tokens total).
Per-core partial kv[b,h,d,e] is AllReduce-summed across cores in bf16 (2 MB),
overlapped with the u-projection; everything else is local.

v2 scheduling (vs the v1 baseline, measured on HW):
  - first-quarter wq/wv DMAs issue before the bulk xT load, so the PE starts
    ~35 us earlier.
  - phase 1 runs quarter-major passes (all 16 q tiles, then all 16 v tiles)
    so the ACT engine loads the Exp/Silu tables twice per quarter instead of
    twice per token tile (128 -> ~9 table loads, 1.3 us each).
  - q is spilled NATURAL and re-loaded transposed in phase 3 via the DMA
    xbar (dma_start_transpose), removing all 256 PE transpose matmuls and
    their PSUM/DVE copies.
  - the kv AllReduce runs in bf16 (verified 7.2e-3 rel err vs 2e-2 budget).
  - Wo streams in during phase 2 (u-projection) so phase 3 starts clean.
  - phase 3 is software-pipelined A(att+stats) / B(bcast) / Z(zT DVE chain) /
    Y(out-proj): each batch's 26 us zT vector chain hides under the previous
    batch's 67 us of output-projection matmuls, and y DMAs straight out of
    PSUM.
All matmuls bf16 (fp32 PSUM accumulate). fp8 was evaluated and rejected:
e4m3 operand noise gives 4-8e-2 rel err vs the 2e-2 gate (measured on the
fixed seed-0 inputs).
"""

import sys

if "/opt/trn_rl_repo" not in sys.path:
    sys.path.insert(0, "/opt/trn_rl_repo")

import numpy as np
import ml_dtypes

B, N, E = 4, 4096, 2048
H_DIM, HEADS, DH = 2048, 16, 128
N_CORES = 8
NL = N // N_CORES          # 512 tokens per (core, batch)
TL = B * NL                # 2048 local tokens per core
ET = E // 128              # 16 contraction tiles
JT = H_DIM // 128          # 16 hidden tiles
NJQ = 4                    # process hidden dim in 4 quarters of 512
TOK_B = NL // 128          # 4 token tiles per batch
LN_EPS = 1e-5

_BUILT = {}


def _build(flags, debug=False):
    import concourse.bacc as bacc
    import concourse.mybir as mybir
    import concourse.tile as tile

    has_bqv, has_bu, has_bo, has_affine = flags
    f32 = mybir.dt.float32
    bf16 = mybir.dt.bfloat16

    nc = bacc.Bacc("TRN2", target_bir_lowering=False, debug=False,
                   num_devices=N_CORES)

    t = {}
    t["xT"] = nc.dram_tensor("xT", [E, TL], bf16, kind="ExternalInput").ap()
    t["wqk"] = nc.dram_tensor("wqk", [E, H_DIM], bf16, kind="ExternalInput").ap()
    t["wv"] = nc.dram_tensor("wv", [E, H_DIM], bf16, kind="ExternalInput").ap()
    t["wu"] = nc.dram_tensor("wu", [E, H_DIM], bf16, kind="ExternalInput").ap()
    t["wo"] = nc.dram_tensor("wo", [H_DIM, E], bf16, kind="ExternalInput").ap()
    if has_bqv:
        t["bqk_r"] = nc.dram_tensor("bqk_r", [1, H_DIM], bf16,
                                    kind="ExternalInput").ap()
        t["bv_r"] = nc.dram_tensor("bv_r", [1, H_DIM], bf16,
                                   kind="ExternalInput").ap()
    if has_bo:
        t["bo_r"] = nc.dram_tensor("bo_r", [1, E], bf16,
                                   kind="ExternalInput").ap()
    if has_bu:
        t["bu_c"] = nc.dram_tensor("bu_c", [128, JT], f32,
                                   kind="ExternalInput").ap()
    if has_affine:
        t["g_c"] = nc.dram_tensor("g_c", [128, JT], f32,
                                  kind="ExternalInput").ap()
        t["b_c"] = nc.dram_tensor("b_c", [128, JT], f32,
                                  kind="ExternalInput").ap()
    t["y"] = nc.dram_tensor("y", [TL, E], f32, kind="ExternalOutput").ap()

    with tile.TileContext(nc) as tc:
        _body(nc, tc, tile, mybir, f32, bf16, t, flags)
    nc.compile()
    return nc


def _body(nc, tc, tile, mybir, f32, bf16, t, flags):
    Act = mybir.ActivationFunctionType
    Alu = mybir.AluOpType
    has_bqv, has_bu, has_bo, has_affine = flags

    with (
        tc.tile_pool(name="consts", bufs=1) as consts,
        tc.tile_pool(name="pre3", bufs=1) as pre3,
        tc.tile_pool(name="dram", bufs=1, space="DRAM") as dram,
    ):
        ones_col = consts.tile([128, 1], bf16)
        nc.vector.memset(ones_col, 1.0)
        ones_row = consts.tile([1, 128], bf16)
        nc.vector.memset(ones_row, 1.0)
        eps_sb = consts.tile([1, 1], f32)
        nc.vector.memset(eps_sb, LN_EPS)
        if has_bqv or has_bo:
            ones_bf = consts.tile([1, 128], bf16)
            nc.vector.memset(ones_bf, 1.0)
        if has_bqv:
            bqk_sb = consts.tile([1, H_DIM], bf16)
            nc.sync.dma_start(bqk_sb[:], t["bqk_r"][:])
            bv_sb = consts.tile([1, H_DIM], bf16)
            nc.sync.dma_start(bv_sb[:], t["bv_r"][:])
        if has_bo:
            bo_sb = consts.tile([1, E], bf16)
            nc.sync.dma_start(bo_sb[:], t["bo_r"][:])
        if has_bu:
            bu_sb = consts.tile([128, JT], f32)
            nc.sync.dma_start(bu_sb[:], t["bu_c"][:])
        if has_affine:
            g_sb = consts.tile([128, JT], f32)
            nc.sync.dma_start(g_sb[:], t["g_c"][:])
            b_sb = consts.tile([128, JT], f32)
            nc.sync.dma_start(b_sb[:], t["b_c"][:])

        q_dram = dram.tile([TL, H_DIM], bf16)
        uT_dram = dram.tile([H_DIM, TL], bf16)
        cc_in = dram.tile([B * HEADS * 128, DH], bf16)
        cc_out = dram.tile([B * HEADS * 128, DH], bf16, addr_space="Shared")

        with tc.tile_pool(name="xt_pool", bufs=1) as xt_pool:
            # w2 spans phases 1+2 (wu prefetch overlaps phase 1)
            w2_ctx = tc.tile_pool(name="w2", bufs=1)
            w2 = w2_ctx.__enter__()

            with (
                tc.tile_pool(name="w1", bufs=1) as w1,
                tc.tile_pool(name="st1", bufs=1) as st1,
                tc.tile_pool(name="ps_proj", bufs=1, space="PSUM") as psp,
                tc.tile_pool(name="ps_kv", bufs=1, space="PSUM") as pskv,
            ):
                # first-quarter weights BEFORE the bulk x load: the PE's
                # first matmul needs wq[jq=0] + xt chunk 0 only
                wq0 = w1.tile([128, ET, 512], bf16, tag="wq", bufs=1)
                nc.sync.dma_start(
                    wq0[:],
                    t["wqk"][:, 0:512].rearrange("(t p) j -> p t j", p=128))
                wv0 = w1.tile([128, ET, 512], bf16, tag="wv", bufs=1)
                nc.sync.dma_start(
                    wv0[:],
                    t["wv"][:, 0:512].rearrange("(t p) j -> p t j", p=128))

                xt = xt_pool.tile([128, ET, TL], bf16)   # 8 MB, ph1+2
                for tt in range(ET):
                    nc.sync.dma_start(
                        xt[:, tt],
                        t["xT"].rearrange("(t p) n -> t p n", p=128)[tt])

                # wu streams as 16 jl-chunks (512 KB, 6-deep rotation):
                # fine WAR granularity spreads the DMAs through phase 2
                # instead of 2 MB bursts that collide with the AllReduce
                wu_pre = []
                for ch in range(6):
                    wu_c = w2.tile([128, ET, 128], bf16, tag="wu", bufs=6)
                    nc.sync.dma_start(
                        wu_c[:],
                        t["wu"][:, ch * 128:(ch + 1) * 128]
                        .rearrange("(t p) j -> p t j", p=128))
                    wu_pre.append(wu_c)

                # ---------------- phase 1: q/v projections + partial kv ----
                for jq in range(NJQ):
                    if jq == 0:
                        wq_sb, wv_sb = wq0, wv0
                    else:
                        wq_sb = w1.tile([128, ET, 512], bf16, tag="wq",
                                        bufs=1)
                        nc.sync.dma_start(
                            wq_sb[:],
                            t["wqk"][:, jq * 512:(jq + 1) * 512]
                            .rearrange("(t p) j -> p t j", p=128))
                        wv_sb = w1.tile([128, ET, 512], bf16, tag="wv",
                                        bufs=1)
                        nc.sync.dma_start(
                            wv_sb[:],
                            t["wv"][:, jq * 512:(jq + 1) * 512]
                            .rearrange("(t p) j -> p t j", p=128))

                    q_bf = st1.tile([128, 16, 512], bf16, tag="qbf", bufs=1)
                    v_bf = st1.tile([128, 16, 512], bf16, tag="vbf", bufs=1)

                    # q pass: 16 token tiles, Exp table loads once
                    for idx in range(16):
                        tok0 = idx * 128
                        q_ps = psp.tile([128, 512], f32, tag="qps", bufs=2)
                        for tt in range(ET):
                            nc.tensor.matmul(
                                q_ps[:], xt[:, tt, tok0:tok0 + 128],
                                wq_sb[:, tt],
                                start=(tt == 0),
                                stop=(not has_bqv and tt == ET - 1))
                        if has_bqv:
                            nc.tensor.matmul(
                                q_ps[:], ones_bf[:],
                                bqk_sb[:, jq * 512:(jq + 1) * 512],
                                start=False, stop=True)
                        # elu(q) = (max(q,0) - 1) + exp(min(q, 0))
                        tmin = st1.tile([128, 512], f32, tag="tmin", bufs=2)
                        nc.vector.tensor_scalar_min(tmin[:], q_ps[:], 0.0)
                        texp = st1.tile([128, 512], bf16, tag="texp", bufs=2)
                        nc.scalar.activation(texp[:], tmin[:], Act.Exp)
                        trelu = st1.tile([128, 512], bf16, tag="trelu",
                                         bufs=2)
                        nc.vector.tensor_scalar(trelu[:], q_ps[:], 0.0,
                                                -1.0, Alu.max, Alu.add)
                        nc.vector.tensor_add(q_bf[:, idx], trelu[:], texp[:])
                        # spill q natural; phase 3 re-loads it transposed
                        nc.sync.dma_start(
                            q_dram[tok0:tok0 + 128,
                                   jq * 512:(jq + 1) * 512],
                            q_bf[:, idx])

                    # v pass: Silu table loads once
                    for idx in range(16):
                        tok0 = idx * 128
                        v_ps = psp.tile([128, 512], f32, tag="vps", bufs=2)
                        for tt in range(ET):
                            nc.tensor.matmul(
                                v_ps[:], xt[:, tt, tok0:tok0 + 128],
                                wv_sb[:, tt],
                                start=(tt == 0),
                                stop=(not has_bqv and tt == ET - 1))
                        if has_bqv:
                            nc.tensor.matmul(
                                v_ps[:], ones_bf[:],
                                bv_sb[:, jq * 512:(jq + 1) * 512],
                                start=False, stop=True)
                        nc.scalar.activation(v_bf[:, idx], v_ps[:], Act.Silu)

                    # kv pass: per-head PSUM accumulation over token tiles
                    for b in range(B):
                        kv_sb = st1.tile([128, 4, DH], bf16, tag="kvsb",
                                         bufs=2)
                        for h in range(4):
                            kv_ps = pskv.tile([128, DH], f32, tag="kv",
                                              bufs=2)
                            for tk in range(TOK_B):
                                idx = b * 4 + tk
                                nc.tensor.matmul(
                                    kv_ps[:],
                                    q_bf[:, idx, h * 128:(h + 1) * 128],
                                    v_bf[:, idx, h * 128:(h + 1) * 128],
                                    start=(tk == 0), stop=(tk == TOK_B - 1))
                            nc.vector.tensor_copy(kv_sb[:, h], kv_ps[:])
                        r0 = (b * HEADS + jq * 4) * 128
                        nc.sync.dma_start(
                            cc_in[r0:r0 + 512, :]
                            .rearrange("(h d) e -> d h e", h=4),
                            kv_sb[:])

            # ---------------- kv AllReduce across the 8 cores (bf16) -------
            nc.gpsimd.collective_compute(
                "AllReduce", mybir.AluOpType.add,
                replica_groups=[list(range(N_CORES))],
                ins=[cc_in.opt()], outs=[cc_out.opt()])

            # ---------------- phase 2: uT projection (overlaps AR) ---------
            # batch 0's qT tiles stream in mid-phase-2 via the Activation
            # DGE queue (the Sync queue is blocked behind phase-2-dependent
            # u spills, and issuing at phase-2 start would contend with the
            # AllReduce's bandwidth peak): pre3 addresses don't overlap any
            # live pool and q_dram is final, so A(0) starts with operands
            # resident.
            qpre = {}

            with (
                tc.tile_pool(name="ps_u", bufs=1, space="PSUM") as psu,
            ):
                for jqu in range(NJQ):
                    if jqu == 2:
                        for h in range(HEADS):
                            qb = pre3.tile([128, NL], bf16, tag="qb",
                                           bufs=24)
                            nc.scalar.dma_start_transpose(
                                qb[:], q_dram[0:NL, h * 128:(h + 1) * 128])
                            qpre[(0, h)] = qb
                    for jl in range(4):
                        jt = jqu * 4 + jl
                        if jt < 6:
                            wu_c = wu_pre[jt]
                        else:
                            wu_c = w2.tile([128, ET, 128], bf16, tag="wu",
                                           bufs=6)
                            nc.sync.dma_start(
                                wu_c[:],
                                t["wu"][:, jt * 128:(jt + 1) * 128]
                                .rearrange("(t p) j -> p t j", p=128))
                        u_ps = psu.tile([128, 4, 512], f32, tag="ups", bufs=2)
                        for tt in range(ET):
                            for c in range(4):
                                nc.tensor.matmul(
                                    u_ps[:, c],
                                    wu_c[:, tt],
                                    xt[:, tt, c * 512:(c + 1) * 512],
                                    start=(tt == 0), stop=(tt == ET - 1))
                        u_st = w2.tile([128, TL], bf16, tag="ust", bufs=1)
                        ubias = bu_sb[:, jt:jt + 1] if has_bu else 0.0
                        for c in range(4):
                            nc.scalar.activation(
                                u_st[:, c * 512:(c + 1) * 512], u_ps[:, c],
                                Act.Silu, bias=ubias)
                        nc.sync.dma_start(
                            uT_dram[jt * 128:(jt + 1) * 128, :], u_st[:])
                # batch 0's kv load: the ACT queue reaches this at the end
                # of the u-silu stream; the AllReduce is long finished
                kvb0 = pre3.tile([128, HEADS, DH], bf16, tag="kvb", bufs=2)
                nc.scalar.dma_start(
                    kvb0[:],
                    cc_out[0:HEADS * 128, :]
                    .rearrange("(h d) e -> d h e", h=HEADS))
            w2_ctx.__exit__(None, None, None)

            # ------------- phase 3: attention, layernorm, output proj ------
            # x is dead after phase 2 and xt has exactly wo's shape
            # [128, 16, 2048] bf16 — stream wo into it (zero extra SBUF).
            # Software pipeline: A(b)=att+stats (interleaved matmuls),
            # B(b)=stat broadcast, Z(b)=zT DVE chain, Y(b)=out-proj.
            # Z(b) hides under Y(b-1)'s matmul stream.
            wo_sb = xt
            with (
                tc.tile_pool(name="st3", bufs=1) as st3,
                tc.tile_pool(name="ps_att", bufs=1, space="PSUM") as psa,
                tc.tile_pool(name="ps_sm", bufs=1, space="PSUM") as pssm,
                tc.tile_pool(name="ps_y", bufs=1, space="PSUM") as psy,
            ):
                state = {}

                def wo_quarter(i):
                    for ct in range(i * 4, i * 4 + 4):
                        nc.sync.dma_start(
                            wo_sb[:, ct],
                            t["wo"].rearrange("(t p) e -> t p e", p=128)[ct])

                def att_block(b):
                    if b == 0:
                        kvb = kvb0
                    else:
                        kvb = pre3.tile([128, HEADS, DH], bf16, tag="kvb",
                                        bufs=2)
                        nc.sync.dma_start(
                            kvb[:],
                            cc_out[b * HEADS * 128:(b + 1) * HEADS * 128, :]
                            .rearrange("(h d) e -> d h e", h=HEADS))
                    att = st3.tile([128, HEADS, NL], bf16, tag="att", bufs=4)
                    sum_ps = pssm.tile([1, NL], f32, tag="sum", bufs=1)
                    ssq_ps = pssm.tile([1, NL], f32, tag="ssq", bufs=1)

                    # per-head: load qT via DMA xbar transpose, att matmul,
                    # then interleave the sum/ssq accumulation matmuls two
                    # heads behind so PE consumption paces the qb DMAs
                    def stats(h):
                        nc.tensor.matmul(sum_ps[:], ones_col[:], att[:, h],
                                         start=(h == 0),
                                         stop=(h == HEADS - 1))
                        sq = st3.tile([128, NL], bf16, tag="sq", bufs=1)
                        nc.vector.tensor_mul(sq[:], att[:, h], att[:, h])
                        nc.tensor.matmul(ssq_ps[:], ones_col[:], sq[:],
                                         start=(h == 0),
                                         stop=(h == HEADS - 1))

                    for h in range(HEADS):
                        if (b, h) in qpre:
                            qb = qpre[(b, h)]
                        else:
                            qb = pre3.tile([128, NL], bf16, tag="qb",
                                           bufs=24)
                            nc.sync.dma_start_transpose(
                                qb[:],
                                q_dram[b * NL:(b + 1) * NL,
                                       h * 128:(h + 1) * 128])
                        att_ps = psa.tile([128, NL], f32, tag="attps",
                                          bufs=2)
                        nc.tensor.matmul(att_ps[:], kvb[:, h], qb[:],
                                         start=True, stop=True)
                        nc.scalar.copy(att[:, h], att_ps[:])
                        if h >= 2:
                            stats(h - 2)
                    stats(HEADS - 2)
                    stats(HEADS - 1)

                    mean = st3.tile([1, NL], f32, tag="mean", bufs=1)
                    nc.vector.tensor_scalar_mul(mean[:], sum_ps[:],
                                                1.0 / H_DIM)
                    msq = st3.tile([1, NL], f32, tag="msq", bufs=1)
                    nc.vector.tensor_scalar_mul(msq[:], ssq_ps[:],
                                                1.0 / H_DIM)
                    m2 = st3.tile([1, NL], f32, tag="m2", bufs=1)
                    nc.vector.tensor_mul(m2[:], mean[:], mean[:])
                    nc.vector.tensor_sub(msq[:], msq[:], m2[:])   # var
                    nc.scalar.activation(m2[:], msq[:], Act.Sqrt,  # std
                                         bias=eps_sb[:])
                    rstd_f = st3.tile([1, NL], f32, tag="rstdf", bufs=1)
                    nc.vector.reciprocal(rstd_f[:], m2[:])
                    rstd = st3.tile([1, NL], bf16, tag="rstd", bufs=1)
                    nc.vector.tensor_copy(rstd[:], rstd_f[:])
                    mr = st3.tile([1, NL], bf16, tag="mr", bufs=1)
                    nc.vector.tensor_mul(mr[:], mean[:], rstd_f[:])
                    state[b] = (att, rstd, mr)

                def bcast_block(b):
                    att, rstd, mr = state[b]
                    # uT is only needed by z_block — loading it here keeps
                    # the phase-2-gated DMA waits off att_block's load path
                    uT_b = []
                    for half in range(2):
                        uh = st3.tile([128, 8, NL], bf16, tag="utb", bufs=2)
                        nc.sync.dma_start(
                            uh[:],
                            uT_dram[half * 1024:(half + 1) * 1024,
                                    b * NL:(b + 1) * NL]
                            .rearrange("(jt p) n -> p jt n", p=128))
                        uT_b.append(uh)
                    bc_ps = pssm.tile([128, 2, NL], f32, tag="bc", bufs=1)
                    nc.tensor.matmul(bc_ps[:, 0], ones_row[:], rstd[:],
                                     start=True, stop=True)
                    nc.tensor.matmul(bc_ps[:, 1], ones_row[:], mr[:],
                                     start=True, stop=True)
                    bc_sb = st3.tile([128, 2, NL], bf16, tag="bcs", bufs=2)
                    nc.vector.tensor_copy(bc_sb[:], bc_ps[:])
                    state[b] = (att, uT_b, bc_sb)

                def z_block(b):
                    # zT overwrites att in place: each att slice is fully
                    # consumed (stats + s1) before its zT write
                    att, uT_b, bc_sb = state[b]
                    for jt in range(JT):
                        s1 = st3.tile([128, NL], bf16, tag="s1", bufs=2)
                        nc.vector.tensor_mul(s1[:], att[:, jt], bc_sb[:, 0])
                        s2 = st3.tile([128, NL], bf16, tag="s2", bufs=2)
                        nc.vector.tensor_sub(s2[:], s1[:], bc_sb[:, 1])
                        if has_affine:
                            s3 = st3.tile([128, NL], bf16, tag="s3", bufs=2)
                            nc.vector.tensor_scalar(
                                s3[:], s2[:], g_sb[:, jt:jt + 1],
                                b_sb[:, jt:jt + 1], Alu.mult, Alu.add)
                        else:
                            s3 = s2
                        nc.vector.tensor_mul(att[:, jt], s3[:],
                                             uT_b[jt // 8][:, jt % 8])
                    state[b] = att

                def y_block(b):
                    zT = state.pop(b)
                    for tsl in range(TOK_B):
                        for eb in range(4):
                            e0 = eb * 512
                            y_ps = psy.tile([128, 512], f32, tag="yps",
                                            bufs=2)
                            for ct in range(JT):
                                nc.tensor.matmul(
                                    y_ps[:],
                                    zT[:, ct, tsl * 128:(tsl + 1) * 128],
                                    wo_sb[:, ct, e0:e0 + 512],
                                    start=(ct == 0),
                                    stop=(not has_bo and ct == JT - 1))
                            if has_bo:
                                nc.tensor.matmul(
                                    y_ps[:], ones_bf[:],
                                    bo_sb[:, e0:e0 + 512],
                                    start=False, stop=True)
                            ybuf = st3.tile([128, 512], f32, tag="ybuf",
                                            bufs=1)
                            nc.scalar.copy(ybuf[:], y_ps[:])
                            nc.sync.dma_start(
                                t["y"][b * NL + tsl * 128:
                                       b * NL + (tsl + 1) * 128,
                                       e0:e0 + 512],
                                ybuf[:])

                att_block(0)
                att_block(1)
                bcast_block(0)
                z_block(0)
                att_block(2)
                wo_quarter(0)
                bcast_block(1)
                z_block(1)
                wo_quarter(1)
                wo_quarter(2)
                wo_quarter(3)
                bcast_block(2)
                y_block(0)
                z_block(2)
                att_block(3)
                y_block(1)
                bcast_block(3)
                z_block(3)
                y_block(2)
                y_block(3)


def _get_nc(flags, debug=False):
    key = (flags, debug)
    if key not in _BUILT:
        _BUILT[key] = _build(flags, debug)
    return _BUILT[key]


def make_in_maps(x, Wqk, bqk, Wv, bv, Wu, bu, Wo, bo, ln_g, ln_b):
    bf16 = ml_dtypes.bfloat16
    f32 = np.float32
    x = np.asarray(x)
    flags = (
        bool(np.any(bqk) or np.any(bv)),
        bool(np.any(bu)),
        bool(np.any(bo)),
        bool(np.any(np.asarray(ln_g) != 1.0) or np.any(ln_b)),
    )
    shared = {
        "wqk": np.asarray(Wqk, f32).astype(bf16),
        "wv": np.asarray(Wv, f32).astype(bf16),
        "wu": np.asarray(Wu, f32).astype(bf16),
        "wo": np.asarray(Wo, f32).astype(bf16),
    }
    if flags[0]:
        shared["bqk_r"] = np.asarray(bqk, f32).astype(bf16).reshape(1, H_DIM)
        shared["bv_r"] = np.asarray(bv, f32).astype(bf16).reshape(1, H_DIM)
    if flags[1]:
        shared["bu_c"] = np.ascontiguousarray(
            np.asarray(bu, f32).reshape(JT, 128).T)
    if flags[2]:
        shared["bo_r"] = np.asarray(bo, f32).astype(bf16).reshape(1, E)
    if flags[3]:
        shared["g_c"] = np.ascontiguousarray(
            np.asarray(ln_g, f32).reshape(JT, 128).T)
        shared["b_c"] = np.ascontiguousarray(
            np.asarray(ln_b, f32).reshape(JT, 128).T)
    in_maps = []
    for c in range(N_CORES):
        xc = np.ascontiguousarray(
            x[:, c * NL:(c + 1) * NL, :].reshape(TL, E).T).astype(bf16)
        in_maps.append({"xT": xc, **shared})
    return flags, in_maps


def kernel(x, Wqk, bqk, Wv, bv, Wu, bu, Wo, bo, ln_g, ln_b, **_unused):
    from concourse.bass_utils import run_bass_kernel_spmd

    flags, in_maps = make_in_maps(x, Wqk, bqk, Wv, bv, Wu, bu, Wo, bo,
                                  ln_g, ln_b)
    nc = _get_nc(flags)
    res = run_bass_kernel_spmd(nc, in_maps, core_ids=list(range(N_CORES)))

    y = np.empty((B, N, E), np.float32)
    for c in range(N_CORES):
        y[:, c * NL:(c + 1) * NL, :] = res.results[c]["y"].reshape(B, NL, E)
    return y
